# revision 35
# baseline (speedup 1.0000x reference)
"""Trainium2 Bass kernel for chunked causal linear attention (elu+1 feature map).

Reference computation (B=2, N=2048, D=1024, DHAT=512, H=16, F=32, G=64):
    Q = phi(x @ wq + bq), K = phi(x @ wk + bk), V = x @ wv + bv   (per-head split)
    kv_t = cumsum_t(K_t outer V_t);  Z_t = 1/(Q_t . cumsum_t(K)_t + 1e-6)
    out_t = (Q_t . kv_t) * Z_t;  y = out @ wo + bo
with phi(u) = elu(u) + 1 = min(exp(u), max(u + 1, 1)).

Sharding over 8 cores: core c handles batch b = c//4 and heads 4r..4r+3
(r = c%4).  Each core projects its head slice, runs chunk-parallel linear
attention (chunk C=256), and computes a PARTIAL output projection through its
256 rows of wo; the host sums the 4 bf16 partials per batch plus bo.

Layout strategy (v2):
  * Projection phase builds, per 512-t block: feature-major Q^T/K^T (for the
    intra-chunk A matmuls), and t-major [V|K|ones] 128-t blocks in one
    persistent SBUF tile vkT (V projected DIRECTLY t-major with bias folded
    in as a rank-1 matmul; K transposed 4-heads-at-a-time on the PE).
  * Attention is computed t-major: per 128-t block the PE produces
    o[t, 4 x (V,.,den)] so the denominator lands as a per-partition column;
    normalize is then a per-partition-scale ACT op (no PE broadcast, no wide
    DVE reciprocal/multiply).  The normalized attn is transposed back to
    feature-major for the output projection (2 PE transposes / 128-t block).
  * Running state S = cumsum K^T[V|K|ones] stays RESIDENT IN PSUM (PE
    accumulates across chunks); only a [128,128] bf16 cast per chunk runs on
    the DVE.
  * y partials are written bf16 (halves output DMA).
"""
import os
import sys
import types

sys.path.insert(0, "/opt/trn_rl_repo")

import ml_dtypes
import numpy as np

# ---- problem constants (hardcoded; kernel.py must be self-contained) ----
B, N, D, DHAT, H = 2, 2048, 1024, 512, 16
F = DHAT // H        # 32
G = D // H           # 64
NCORES = 8
CHUNK = 256          # attention chunk along t
NCHUNK = N // CHUNK  # 8
SB = 128             # s-block (128-t block)
NSB = N // SB        # 16
TB = 512             # projection t-block
KD = D // 128        # 8 contraction tiles
BF16NP = ml_dtypes.bfloat16


def _install_ntff_hook():
    """Register the axon NTFF profiling hook (stub antenv lacks axon_hooks)."""
    if "antenv.axon_hooks" in sys.modules:
        return
    try:
        from trn_agent_boot.trn_boot import _ntff_profile_via_ctypes
        hook = _ntff_profile_via_ctypes("/opt/axon/libaxon_pjrt.so")
    except Exception:
        hook = None
    m = types.ModuleType("antenv.axon_hooks")
    m.get_axon_ntff_profile_hook = lambda: hook
    m.set_axon_ntff_profile_hook = lambda h: None
    sys.modules["antenv.axon_hooks"] = m


def build_nc():
    import concourse.bass as bass
    import concourse.mybir as mybir
    import concourse.tile as tile
    from concourse import bacc

    F32 = mybir.dt.float32
    BF16 = mybir.dt.bfloat16
    AF = mybir.ActivationFunctionType
    ALU = mybir.AluOpType

    nc = bacc.Bacc("TRN2", target_bir_lowering=False, debug=False,
                   num_devices=NCORES)

    # ---- per-core DRAM parameters (bf16 operands, pre-tiled on host) ----
    xT_e = nc.declare_dram_parameter("xT", [N // TB, KD, 128, TB],
                                     BF16, isOutput=False)
    wq_e = nc.declare_dram_parameter("wq", [KD, 128, 4 * F], BF16,
                                     isOutput=False)
    wk_e = nc.declare_dram_parameter("wk", [KD, 128, 4 * F], BF16,
                                     isOutput=False)
    wv_e = nc.declare_dram_parameter("wv", [KD * 2, 128, 128], BF16,
                                     isOutput=False)
    wo_e = nc.declare_dram_parameter("wo", [2, 128, D], BF16, isOutput=False)
    bq_e = nc.declare_dram_parameter("bq", [4 * F, 1], F32, isOutput=False)
    bk_e = nc.declare_dram_parameter("bk", [4 * F, 1], F32, isOutput=False)
    bv_e = nc.declare_dram_parameter("bv", [1, 4 * G], BF16, isOutput=False)
    y_e = nc.declare_dram_parameter("y", [N // 128, 2, 128, 512], BF16,
                                    isOutput=True)

    # causal mask [triu(s0 vs t) | triu(s1 vs t-high)] for one 256-chunk:
    # cols 0:256 mask block0 [s0, t 0:256]; cols 256:384 mask block1
    # [s1, t 128:256]
    m0 = np.zeros((128, CHUNK), np.float32)
    tri = np.zeros((128, 128), np.float32)
    for s in range(128):
        m0[s, s:] = 1.0
        tri[s, s:] = 1.0
    mask1 = np.concatenate([m0, tri], axis=1)          # [128, 384]
    maskc_d = nc.inline_tensor(
        np.concatenate([mask1, mask1], axis=1).astype(BF16NP), "maskc")
    ident_d = nc.inline_tensor(np.eye(128, dtype=np.float32).astype(BF16NP),
                               "identc")
    ones1_d = nc.inline_tensor(np.ones((1, 128), np.float32).astype(BF16NP),
                               "ones1c")
    onesr_d = nc.inline_tensor(np.ones((1, 64), np.float32), "onesrc")

    with tile.TileContext(nc) as tc:
        with (
            tc.tile_pool(name="persist", bufs=1) as pers,
            tc.tile_pool(name="xin", bufs=4) as xin,
            tc.tile_pool(name="ppool", bufs=2, space="PSUM") as pp,
            tc.tile_pool(name="avpool", bufs=2, space="PSUM") as av,
            tc.tile_pool(name="opool", bufs=2, space="PSUM") as op,
            tc.tile_pool(name="bcpool", bufs=1, space="PSUM") as bcp,
            tc.tile_pool(name="spool", bufs=1, space="PSUM") as sp,
            tc.tile_pool(name="work", bufs=6) as work,
            tc.tile_pool(name="sbf16", bufs=2) as sbf16p,
            tc.tile_pool(name="ysb", bufs=3) as ysb,
        ):
            # ---- persistent SBUF tiles ----
            wq_sb = pers.tile([128, KD * 128], BF16)
            wk_sb = pers.tile([128, KD * 128], BF16)
            wv_sb = pers.tile([128, KD * 256], BF16)
            wo_sb = pers.tile([128, 2 * D], BF16)
            bq_sb = pers.tile([4 * F, 1], F32)
            bk_sb = pers.tile([4 * F, 1], F32)
            bv_sb = pers.tile([1, 4 * G], BF16)
            ones1 = pers.tile([1, 128], BF16)
            onesr = pers.tile([1, 64], mybir.dt.float32r)
            ident = pers.tile([128, 128], BF16)
            maskc_sb = pers.tile([128, 768], BF16)
            qT = pers.tile([128, N], BF16)
            kT = pers.tile([128, N], BF16)
            # t-major [V|K|ones] blocks: s-block j at cols 512j, head h at
            # 512j+128h: [V(0:64) | K(64:96) | ones(96) | zero(97:128)]
            vkT = pers.tile([128, NSB * 512], BF16)
            # feature-major normalized attn: j-half jh (heads 2jh,2jh+1) at
            # cols jh*N + t
            aT = pers.tile([128, 2 * N], BF16)

            # zero vkT, then set the ones columns (col 96 of each block)
            nc.vector.memset(vkT[:], 0.0)
            nc.vector.memset(
                bass.AP(tensor=vkT.tensor, offset=vkT.offset + 96,
                        ap=[vkT.ap[0], [128, 4 * NSB]]), 1.0)

            # ---- startup DMAs: interleave weight/x issue on 2 engines so
            # the first projection matmul starts ~2us in ----
            # sync engine: x t-block 0 (2 halves), ident, x prefetches, wo
            xt_tiles = []
            xt0 = xin.tile([128, KD * TB], BF16, tag="xt", name="xt0")
            for q in range(4):
                nc.sync.dma_start(
                    xt0[:, 2 * q * TB:2 * (q + 1) * TB].rearrange(
                        "p (k c) -> p k c", k=2),
                    xT_e[0, 2 * q:2 * q + 2].rearrange("k p c -> p k c"))
            xt_tiles.append(xt0)
            nc.sync.dma_start(ident[:], ident_d[:])
            # wv rides both DMA queues (half each) so the first V-bundle
            # isn't starved behind the other weight transfers
            nc.sync.dma_start(
                wv_sb[:, 8 * 128:].rearrange("p (km c) -> p km c", km=KD),
                wv_e[8:].rearrange("km p c -> p km c"))
            # scalar engine: weights + biases (issue in first-use order)
            for hh in range(2):
                nc.scalar.dma_start(
                    wq_sb[:, 512 * hh:512 * (hh + 1)].rearrange(
                        "p (k c) -> p k c", k=4),
                    wq_e[4 * hh:4 * (hh + 1)].rearrange("k p c -> p k c"))
            nc.scalar.dma_start(
                wk_sb[:].rearrange("p (k c) -> p k c", k=KD),
                wk_e[:].rearrange("k p c -> p k c"))
            nc.scalar.dma_start(bq_sb[:], bq_e[:])
            nc.scalar.dma_start(bk_sb[:], bk_e[:])
            nc.scalar.dma_start(
                wv_sb[:, 0:8 * 128].rearrange("p (km c) -> p km c", km=KD),
                wv_e[0:8].rearrange("km p c -> p km c"))
            nc.scalar.dma_start(bv_sb[:], bv_e[:])
            nc.scalar.dma_start(ones1[:], ones1_d[:])
            nc.scalar.dma_start(onesr[:], onesr_d[:].bitcast(mybir.dt.float32r))

            # remaining prefetches (sync engine), consts needed by chunk 0
            # issued before the later x blocks
            xt1 = xin.tile([128, KD * TB], BF16, tag="xt", name="xt1")
            nc.sync.dma_start(
                xt1[:].rearrange("p (k c) -> p k c", k=KD),
                xT_e[1].rearrange("k p c -> p k c"))
            xt_tiles.append(xt1)
            nc.sync.dma_start(maskc_sb[:], maskc_d[:])
            nc.sync.dma_start(
                wo_sb[:].rearrange("p (j c) -> p j c", j=2),
                wo_e[:].rearrange("j p c -> p j c"))
            for tb in range(2, N // TB):
                xt = xin.tile([128, KD * TB], BF16, tag="xt", name="xt")
                nc.sync.dma_start(
                    xt[:].rearrange("p (k c) -> p k c", k=KD),
                    xT_e[tb].rearrange("k p c -> p k c"))
                xt_tiles.append(xt)

            def vk_blk(jj, h, c0=0, c1=128):
                return bass.AP(
                    tensor=vkT.tensor,
                    offset=vkT.offset + 512 * jj + 128 * h + c0,
                    ap=[vkT.ap[0], [1, c1 - c0]])

            # ---- projection phase, emitted as bundles so t-blocks 1..3
            # interleave into the attention chunks (their fat N=512 streams
            # keep the PE HAM clock-gate at full rate) ----
            def qk_bundle(tb, w_sb, b_sb, dstT):
                tsl = slice(TB * tb, TB * (tb + 1))
                xt = xt_tiles[tb]
                ps = pp.tile([128, TB], F32, tag="p", name="qk_ps")
                for k in range(KD):
                    nc.tensor.matmul(ps[:], w_sb[:, 128 * k:128 * (k + 1)],
                                     xt[:, TB * k:TB * (k + 1)],
                                     start=(k == 0), stop=(k == KD - 1))
                # phi(u) = min(exp(u), relu(u) + 1), bias folded in
                e_sb = work.tile([128, TB], BF16, tag="phi_e", name="phi_e")
                nc.scalar.activation(e_sb[:], ps[:], AF.Exp, bias=b_sb[:])
                r_sb = work.tile([128, TB], BF16, tag="phi_r", name="phi_r")
                nc.scalar.activation(r_sb[:], ps[:], AF.Relu, bias=b_sb[:])
                nc.vector.scalar_tensor_tensor(
                    dstT[:, tsl], r_sb[:], 1.0, e_sb[:],
                    op0=ALU.add, op1=ALU.min)

            def v_bundle(tb, j):
                xt = xt_tiles[tb]
                jj = (TB // SB) * tb + j
                vps = av.tile([128, 256], F32, tag="av", name="v_ps")
                nc.tensor.matmul(vps[:], ones1[:], bv_sb[:],
                                 start=True, stop=False)
                for k in range(KD):
                    nc.tensor.matmul(
                        vps[:],
                        xt[:, TB * k + SB * j:TB * k + SB * (j + 1)],
                        wv_sb[:, 256 * k:256 * (k + 1)],
                        start=False, stop=(k == KD - 1))
                # V psum [128t, 4h x 64g] -> vkT V columns (strided)
                vdst = bass.AP(tensor=vkT.tensor,
                               offset=vkT.offset + 512 * jj,
                               ap=[vkT.ap[0], [128, 4], [1, G]])
                vsrc = bass.AP(tensor=vps.tensor, offset=vps.offset,
                               ap=[vps.ap[0], [G, 4], [1, G]])
                nc.scalar.activation(vdst, vsrc, AF.Copy)
                # K^T 4-heads one transpose -> vkT K columns (strided)
                ktp = av.tile([128, 128], BF16, tag="av", name="kt_ps")
                nc.tensor.transpose(ktp[:], kT[:, SB * jj:SB * (jj + 1)],
                                    ident[:])
                kdst = bass.AP(tensor=vkT.tensor,
                               offset=vkT.offset + 512 * jj + G,
                               ap=[vkT.ap[0], [128, 4], [1, F]])
                ksrc = bass.AP(tensor=ktp.tensor, offset=ktp.offset,
                               ap=[ktp.ap[0], [F, 4], [1, F]])
                nc.vector.tensor_copy(kdst, ksrc)

            def proj_bundles(tb):
                return [
                    lambda tb=tb: qk_bundle(tb, wq_sb, bq_sb, qT),
                    lambda tb=tb: qk_bundle(tb, wk_sb, bk_sb, kT),
                    lambda tb=tb, j=0: v_bundle(tb, j),
                    lambda tb=tb, j=1: v_bundle(tb, j),
                    lambda tb=tb, j=2: v_bundle(tb, j),
                    lambda tb=tb, j=3: v_bundle(tb, j),
                ]

            def junk_warm():
                # dummy fat matmul: keeps the PE MAC duty over the HAM
                # un-throttle threshold in chunks with no projection left
                jk = bcp.tile([128, 512], F32, tag="bc", name="jk_ps")
                nc.tensor.matmul(jk[:], ident[:], qT[:, 0:512],
                                 start=True, stop=True)

            # ---- attention, feature-major output, head-pair banks ----
            s_bf = [None]     # bf16 copy of running state (all 4 heads)
            s_all = sp.tile([128, 128], F32, tag="s", name="s_all")

            def chunk_attn(i, fill=()):
                fill = list(fill)
                t0 = CHUNK * i
                csl = slice(t0, t0 + CHUNK)
                # intra-chunk A^T = K_s . Q_t, masked, per head
                ams = []
                for h in range(4):
                    fsl = slice(32 * h, 32 * (h + 1))
                    tp = (32 * h, 0) if h == 3 else None
                    a_ps = av.tile([128, 384], F32, tag="av", name="a_ps")
                    nc.tensor.matmul(a_ps[:, 0:CHUNK],
                                     kT[fsl, t0:t0 + 128], qT[fsl, csl],
                                     start=True, stop=True, tile_position=tp)
                    nc.tensor.matmul(a_ps[:, CHUNK:384],
                                     kT[fsl, t0 + 128:t0 + 256],
                                     qT[fsl, t0 + 128:t0 + 256],
                                     start=True, stop=True, tile_position=tp)
                    am = work.tile([128, 384], BF16, tag="am", name="am")
                    nc.vector.tensor_tensor(am[:], a_ps[:],
                                            maskc_sb[:, 0:384], op=ALU.mult)
                    ams.append(am)
                if fill:
                    fill.pop(0)()
                # f-major o per head pair: [V-rows 0:64 | junk | den row 96],
                # head-even at cols 0:256, head-odd at 256:512
                for p in range(2):
                    o_ps = op.tile([128, 512], F32, tag="o", name="o_ps")
                    for hp in range(2):
                        h = 2 * p + hp
                        fsl = slice(32 * h, 32 * (h + 1))
                        am = ams[h]
                        osl = o_ps[:, 256 * hp:256 * (hp + 1)]
                        nc.tensor.matmul(osl[0:97, :], vk_blk(2 * i, h, 0, 97),
                                         am[:, 0:CHUNK],
                                         start=True, stop=False)
                        nc.tensor.matmul(
                            osl[0:97, 128:256], vk_blk(2 * i + 1, h, 0, 97),
                            am[:, CHUNK:384],
                            start=False, stop=(s_bf[0] is None))
                        if s_bf[0] is not None:
                            nc.tensor.matmul(
                                osl[0:97, :], s_bf[0][fsl, 0:97],
                                qT[fsl, csl], start=False, stop=True,
                                tile_position=(32 * h, 0) if h == 3 else None)
                    # normalize both heads: PE-broadcast the den row to 64
                    # rows, reciprocal (PSUM->SBUF), two tensor multiplies
                    den_r = work.tile([1, 512], mybir.dt.float32r, tag="den",
                                      name="den_r")
                    nc.scalar.activation(den_r[:], o_ps[96:97, :], AF.Copy)
                    bc_ps = bcp.tile([64, 512], F32, tag="bc", name="bc_ps")
                    nc.tensor.matmul(bc_ps[:], onesr[:], den_r[:],
                                     start=True, stop=True)
                    rec_bc = work.tile([64, 512], F32, tag="rec", name="rec_bc")
                    nc.vector.reciprocal_approx_fast(rec_bc[:], bc_ps[:])
                    nc.vector.tensor_tensor(
                        aT[0:G, N * p + t0:N * p + t0 + CHUNK],
                        o_ps[0:G, 0:256], rec_bc[:, 0:256], op=ALU.mult)
                    nc.vector.tensor_tensor(
                        aT[G:2 * G, N * p + t0:N * p + t0 + CHUNK],
                        o_ps[0:G, 256:512], rec_bc[:, 256:512], op=ALU.mult)
                    if fill:
                        fill.pop(0)()
                # state S += K_chunk^T [V|K|ones] (PSUM-resident accumulate)
                if i < NCHUNK - 1:
                    for h in range(4):
                        ssl = s_all[32 * h:32 * (h + 1), :]
                        tp = (0, 32 * h) if h == 3 else None
                        nc.tensor.matmul(ssl, vk_blk(2 * i, h, G, G + F),
                                         vk_blk(2 * i, h),
                                         start=(i == 0), stop=False,
                                         skip_group_check=True,
                                         tile_position=tp)
                        nc.tensor.matmul(ssl, vk_blk(2 * i + 1, h, G, G + F),
                                         vk_blk(2 * i + 1, h),
                                         start=False, stop=True,
                                         skip_group_check=True,
                                         tile_position=tp)
                    sb16 = sbf16p.tile([128, 128], BF16, tag="sbf",
                                       name="sb16")
                    nc.vector.tensor_copy(sb16[:], s_all[:])
                    s_bf[0] = sb16
                for f in fill:
                    f()

            def yproj_half(i, half):
                # output projection for one 128-t tile of chunk i
                tt = 2 * i + half
                tsl = slice(128 * tt, 128 * (tt + 1))
                for eb in range(2):
                    esl = slice(512 * eb, 512 * (eb + 1))
                    y_ps = pp.tile([128, 512], F32, tag="p", name="y_ps")
                    for j in range(2):
                        nc.tensor.matmul(
                            y_ps[:], aT[:, N * j:N * (j + 1)][:, tsl],
                            wo_sb[:, D * j:D * j + D][:, esl],
                            start=(j == 0), stop=(j == 1))
                    y_sb = ysb.tile([128, 512], BF16, tag="ysb",
                                    name="y_sb")
                    if (tt + eb) % 2 == 0:
                        nc.scalar.activation(y_sb[:], y_ps[:], AF.Copy)
                    else:
                        nc.vector.tensor_copy(y_sb[:], y_ps[:])
                    nc.sync.dma_start(y_e[tt, eb], y_sb[:])

            # ---- schedule: minimal projections up front; the rest of the
            # projection bundles, the deferred yproj halves and junk warmers
            # fill slots inside each chunk so every HAM window keeps fat
            # N=512 PE streams.  Chunk i consumes vk s-blocks 2i,2i+1 which
            # its predecessors' V-bundles produce; a-matmuls of chunk i need
            # the K/Q bundles of t-block i//2 (emitted >= 1 chunk earlier).
            B_ = {}
            for tb in range(4):
                bl = proj_bundles(tb)
                for nm, bb in zip(("Q", "K", "V0", "V1", "V2", "V3"), bl):
                    B_[f"{nm}{tb}"] = bb

            def yh(i, half):
                return lambda: yproj_half(i, half)

            for nm in ("Q0", "K0", "V00", "V10"):
                B_[nm]()
            fills = [
                ["V20", "V30", "Q1", junk_warm],
                ["K1", "V01", "V11", yh(0, 0), yh(0, 1)],
                ["V21", "V31", "Q2", yh(1, 0), yh(1, 1)],
                ["K2", "V02", "V12", yh(2, 0), yh(2, 1)],
                ["V22", "V32", "Q3", yh(3, 0), yh(3, 1)],
                ["K3", "V03", "V13", yh(4, 0), yh(4, 1)],
                ["V23", "V33", junk_warm, yh(5, 0), yh(5, 1)],
                [junk_warm, junk_warm, yh(6, 0), yh(6, 1), junk_warm],
            ]
            for i in range(NCHUNK):
                fl = [B_[f] if isinstance(f, str) else f for f in fills[i]]
                chunk_attn(i, fill=fl)
            yproj_half(7, 0)
            yproj_half(7, 1)

    nc.compile()
    return nc


def make_in_maps(x, wq, bq, wk, bk, wv, bv, wo, bo):
    x = np.asarray(x, np.float32)
    in_maps = []
    for c in range(NCORES):
        b, r = divmod(c, 4)
        xt_b = x[b].T.astype(BF16NP)                    # (D, N)
        xt_tiled = np.ascontiguousarray(
            xt_b.reshape(D // 128, 128, N // TB, TB).transpose(2, 0, 1, 3))
        wq_s = np.asarray(wq)[:, 128 * r:128 * (r + 1)].astype(BF16NP)
        wk_s = np.asarray(wk)[:, 128 * r:128 * (r + 1)].astype(BF16NP)
        wv_s = np.asarray(wv)[:, 256 * r:256 * (r + 1)].astype(BF16NP)
        wo_s = np.asarray(wo)[256 * r:256 * (r + 1), :].astype(BF16NP)
        in_maps.append({
            "xT": xt_tiled,
            "wq": np.ascontiguousarray(wq_s.reshape(D // 128, 128, 4 * F)),
            "wk": np.ascontiguousarray(wk_s.reshape(D // 128, 128, 4 * F)),
            "wv": np.ascontiguousarray(
                wv_s.reshape(D // 128, 128, 2, 128).transpose(0, 2, 1, 3)
            ).reshape(D // 128 * 2, 128, 128),
            "wo": np.ascontiguousarray(wo_s.reshape(2, 128, D)),
            "bq": np.ascontiguousarray(
                np.asarray(bq)[128 * r:128 * (r + 1)],
                dtype=np.float32).reshape(-1, 1),
            "bk": np.ascontiguousarray(
                np.asarray(bk)[128 * r:128 * (r + 1)],
                dtype=np.float32).reshape(-1, 1),
            "bv": np.ascontiguousarray(
                np.asarray(bv)[256 * r:256 * (r + 1)].astype(BF16NP)
            ).reshape(1, -1),
        })
    return in_maps


def assemble(results, bo):
    y = np.zeros((B, N, D), np.float32)
    for c in range(NCORES):
        yt = results[c]["y"].astype(np.float32)   # (N//128, 2, 128, 512)
        y[c // 4] += yt.transpose(0, 2, 1, 3).reshape(N, D)
    return y + np.asarray(bo, np.float32).reshape(1, 1, D)


_NC_CACHE = {}


def run(inputs, trace=False):
    _install_ntff_hook()
    from concourse.bass_utils import run_bass_kernel_spmd
    if "nc" not in _NC_CACHE:
        _NC_CACHE["nc"] = build_nc()
    nc = _NC_CACHE["nc"]
    in_maps = make_in_maps(**inputs)
    res = run_bass_kernel_spmd(nc, in_maps, core_ids=list(range(NCORES)),
                               trace=trace)
    return assemble(res.results, inputs["bo"]), res.exec_time_ns


def kernel(**inputs) -> np.ndarray:
    y, _ = run(inputs, trace=False)
    return y


# revision 36
# speedup vs baseline: 1.1459x; 1.1459x over previous
"""Trainium2 Bass kernel for chunked causal linear attention (elu+1 feature map).

Reference computation (B=2, N=2048, D=1024, DHAT=512, H=16, F=32, G=64):
    Q = phi(x @ wq + bq), K = phi(x @ wk + bk), V = x @ wv + bv   (per-head split)
    kv_t = cumsum_t(K_t outer V_t);  Z_t = 1/(Q_t . cumsum_t(K)_t + 1e-6)
    out_t = (Q_t . kv_t) * Z_t;  y = out @ wo + bo
with phi(u) = elu(u) + 1 = min(exp(u), max(u + 1, 1)).

Sharding over 8 cores: core c handles batch b = c//4 and heads 4r..4r+3
(r = c%4).  Each core projects its head slice, runs chunk-parallel linear
attention (chunk C=256), and computes a PARTIAL output projection through its
256 rows of wo; the host sums the 4 bf16 partials per batch plus bo.

Layout strategy (v2):
  * Projection phase builds, per 512-t block: feature-major Q^T/K^T (for the
    intra-chunk A matmuls), and t-major [V|K|ones] 128-t blocks in one
    persistent SBUF tile vkT (V projected DIRECTLY t-major with bias folded
    in as a rank-1 matmul; K transposed 4-heads-at-a-time on the PE).
  * Attention is computed t-major: per 128-t block the PE produces
    o[t, 4 x (V,.,den)] so the denominator lands as a per-partition column;
    normalize is then a per-partition-scale ACT op (no PE broadcast, no wide
    DVE reciprocal/multiply).  The normalized attn is transposed back to
    feature-major for the output projection (2 PE transposes / 128-t block).
  * Running state S = cumsum K^T[V|K|ones] stays RESIDENT IN PSUM (PE
    accumulates across chunks); only a [128,128] bf16 cast per chunk runs on
    the DVE.
  * y partials are written bf16 (halves output DMA).
"""
import os
import sys
import types

sys.path.insert(0, "/opt/trn_rl_repo")

import ml_dtypes
import numpy as np

# ---- problem constants (hardcoded; kernel.py must be self-contained) ----
B, N, D, DHAT, H = 2, 2048, 1024, 512, 16
F = DHAT // H        # 32
G = D // H           # 64
NCORES = 8
CHUNK = 256          # attention chunk along t
NCHUNK = N // CHUNK  # 8
SB = 128             # s-block (128-t block)
NSB = N // SB        # 16
TB = 512             # projection t-block
KD = D // 128        # 8 contraction tiles
BF16NP = ml_dtypes.bfloat16


def _install_ntff_hook():
    """Register the axon NTFF profiling hook (stub antenv lacks axon_hooks)."""
    if "antenv.axon_hooks" in sys.modules:
        return
    try:
        from trn_agent_boot.trn_boot import _ntff_profile_via_ctypes
        hook = _ntff_profile_via_ctypes("/opt/axon/libaxon_pjrt.so")
    except Exception:
        hook = None
    m = types.ModuleType("antenv.axon_hooks")
    m.get_axon_ntff_profile_hook = lambda: hook
    m.set_axon_ntff_profile_hook = lambda h: None
    sys.modules["antenv.axon_hooks"] = m


def build_nc():
    import concourse.bass as bass
    import concourse.mybir as mybir
    import concourse.tile as tile
    from concourse import bacc

    F32 = mybir.dt.float32
    BF16 = mybir.dt.bfloat16
    AF = mybir.ActivationFunctionType
    ALU = mybir.AluOpType

    nc = bacc.Bacc("TRN2", target_bir_lowering=False, debug=False,
                   num_devices=NCORES)

    # ---- per-core DRAM parameters (bf16 operands, pre-tiled on host) ----
    xT_e = nc.declare_dram_parameter("xT", [N // TB, KD, 128, TB],
                                     BF16, isOutput=False)
    wq_e = nc.declare_dram_parameter("wq", [KD, 128, 4 * F], BF16,
                                     isOutput=False)
    wk_e = nc.declare_dram_parameter("wk", [KD, 128, 4 * F], BF16,
                                     isOutput=False)
    wv_e = nc.declare_dram_parameter("wv", [KD * 2, 128, 128], BF16,
                                     isOutput=False)
    wo_e = nc.declare_dram_parameter("wo", [2, 128, D], BF16, isOutput=False)
    bq_e = nc.declare_dram_parameter("bq", [4 * F, 1], F32, isOutput=False)
    bk_e = nc.declare_dram_parameter("bk", [4 * F, 1], F32, isOutput=False)
    bv_e = nc.declare_dram_parameter("bv", [1, 4 * G], BF16, isOutput=False)
    y_e = nc.declare_dram_parameter("y", [N // 128, 2, 128, 512], BF16,
                                    isOutput=True)

    # causal mask [triu(s0 vs t) | triu(s1 vs t-high)] for one 256-chunk:
    # cols 0:256 mask block0 [s0, t 0:256]; cols 256:384 mask block1
    # [s1, t 128:256]
    m0 = np.zeros((128, CHUNK), np.float32)
    tri = np.zeros((128, 128), np.float32)
    for s in range(128):
        m0[s, s:] = 1.0
        tri[s, s:] = 1.0
    mask1 = np.concatenate([m0, tri], axis=1)          # [128, 384]
    maskc_d = nc.inline_tensor(
        np.concatenate([mask1, mask1], axis=1).astype(BF16NP), "maskc")
    ident_d = nc.inline_tensor(np.eye(128, dtype=np.float32).astype(BF16NP),
                               "identc")
    ones1_d = nc.inline_tensor(np.ones((1, 128), np.float32).astype(BF16NP),
                               "ones1c")
    onesr_d = nc.inline_tensor(np.ones((1, 64), np.float32), "onesrc")

    with tile.TileContext(nc) as tc:
        with (
            tc.tile_pool(name="persist", bufs=1) as pers,
            tc.tile_pool(name="xin", bufs=4) as xin,
            tc.tile_pool(name="ppool", bufs=2, space="PSUM") as pp,
            tc.tile_pool(name="avpool", bufs=2, space="PSUM") as av,
            tc.tile_pool(name="opool", bufs=2, space="PSUM") as op,
            tc.tile_pool(name="bcpool", bufs=1, space="PSUM") as bcp,
            tc.tile_pool(name="spool", bufs=1, space="PSUM") as sp,
            tc.tile_pool(name="work", bufs=6) as work,
            tc.tile_pool(name="sbf16", bufs=2) as sbf16p,
            tc.tile_pool(name="ysb", bufs=3) as ysb,
        ):
            # ---- persistent SBUF tiles ----
            wq_sb = pers.tile([128, KD * 128], BF16)
            wk_sb = pers.tile([128, KD * 128], BF16)
            wv_sb = pers.tile([128, KD * 256], BF16)
            wo_sb = pers.tile([128, 2 * D], BF16)
            bq_sb = pers.tile([4 * F, 1], F32)
            bk_sb = pers.tile([4 * F, 1], F32)
            bv_sb = pers.tile([1, 4 * G], BF16)
            ones1 = pers.tile([1, 128], BF16)
            onesr = pers.tile([1, 64], mybir.dt.float32r)
            ident = pers.tile([128, 128], BF16)
            maskc_sb = pers.tile([128, 768], BF16)
            qT = pers.tile([128, N], BF16)
            kT = pers.tile([128, N], BF16)
            # t-major [V|K|ones] blocks: s-block j at cols 512j, head h at
            # 512j+128h: [V(0:64) | K(64:96) | ones(96) | zero(97:128)]
            vkT = pers.tile([128, NSB * 512], BF16)
            # feature-major normalized attn: j-half jh (heads 2jh,2jh+1) at
            # cols jh*N + t
            aT = pers.tile([128, 2 * N], BF16)

            # zero vkT, then set the ones columns (col 96 of each block)
            nc.vector.memset(vkT[:], 0.0)
            nc.vector.memset(
                bass.AP(tensor=vkT.tensor, offset=vkT.offset + 96,
                        ap=[vkT.ap[0], [128, 4 * NSB]]), 1.0)

            # ---- startup DMAs: interleave weight/x issue on 2 engines so
            # the first projection matmul starts ~2us in ----
            # sync engine: x t-block 0 (2 halves), ident, x prefetches, wo
            xt_tiles = []
            xt0 = xin.tile([128, KD * TB], BF16, tag="xt", name="xt0")
            for q in range(4):
                nc.sync.dma_start(
                    xt0[:, 2 * q * TB:2 * (q + 1) * TB].rearrange(
                        "p (k c) -> p k c", k=2),
                    xT_e[0, 2 * q:2 * q + 2].rearrange("k p c -> p k c"))
            xt_tiles.append(xt0)
            nc.sync.dma_start(ident[:], ident_d[:])
            # scalar engine: weights + biases (issue in first-use order)
            for hh in range(2):
                nc.scalar.dma_start(
                    wq_sb[:, 512 * hh:512 * (hh + 1)].rearrange(
                        "p (k c) -> p k c", k=4),
                    wq_e[4 * hh:4 * (hh + 1)].rearrange("k p c -> p k c"))
            nc.scalar.dma_start(
                wk_sb[:].rearrange("p (k c) -> p k c", k=KD),
                wk_e[:].rearrange("k p c -> p k c"))
            nc.scalar.dma_start(bq_sb[:], bq_e[:])
            nc.scalar.dma_start(bk_sb[:], bk_e[:])
            nc.scalar.dma_start(
                wv_sb[:].rearrange("p (km c) -> p km c", km=KD * 2),
                wv_e[:].rearrange("km p c -> p km c"))
            nc.scalar.dma_start(bv_sb[:], bv_e[:])
            nc.scalar.dma_start(ones1[:], ones1_d[:])
            nc.scalar.dma_start(onesr[:], onesr_d[:].bitcast(mybir.dt.float32r))

            # remaining prefetches (sync engine), consts needed by chunk 0
            # issued before the later x blocks
            xt1 = xin.tile([128, KD * TB], BF16, tag="xt", name="xt1")
            nc.sync.dma_start(
                xt1[:].rearrange("p (k c) -> p k c", k=KD),
                xT_e[1].rearrange("k p c -> p k c"))
            xt_tiles.append(xt1)
            nc.sync.dma_start(maskc_sb[:], maskc_d[:])
            nc.sync.dma_start(
                wo_sb[:].rearrange("p (j c) -> p j c", j=2),
                wo_e[:].rearrange("j p c -> p j c"))
            for tb in range(2, N // TB):
                xt = xin.tile([128, KD * TB], BF16, tag="xt", name="xt")
                nc.sync.dma_start(
                    xt[:].rearrange("p (k c) -> p k c", k=KD),
                    xT_e[tb].rearrange("k p c -> p k c"))
                xt_tiles.append(xt)

            def vk_blk(jj, h, c0=0, c1=128):
                return bass.AP(
                    tensor=vkT.tensor,
                    offset=vkT.offset + 512 * jj + 128 * h + c0,
                    ap=[vkT.ap[0], [1, c1 - c0]])

            # ---- projection phase, emitted as bundles so t-blocks 1..3
            # interleave into the attention chunks (their fat N=512 streams
            # keep the PE HAM clock-gate at full rate) ----
            def qk_bundle(tb, w_sb, b_sb, dstT):
                tsl = slice(TB * tb, TB * (tb + 1))
                xt = xt_tiles[tb]
                ps = pp.tile([128, TB], F32, tag="p", name="qk_ps")
                for k in range(KD):
                    nc.tensor.matmul(ps[:], w_sb[:, 128 * k:128 * (k + 1)],
                                     xt[:, TB * k:TB * (k + 1)],
                                     start=(k == 0), stop=(k == KD - 1))
                # phi(u) = min(exp(u), relu(u) + 1), bias folded in
                e_sb = work.tile([128, TB], BF16, tag="phi_e", name="phi_e")
                nc.scalar.activation(e_sb[:], ps[:], AF.Exp, bias=b_sb[:])
                r_sb = work.tile([128, TB], BF16, tag="phi_r", name="phi_r")
                nc.scalar.activation(r_sb[:], ps[:], AF.Relu, bias=b_sb[:])
                nc.vector.scalar_tensor_tensor(
                    dstT[:, tsl], r_sb[:], 1.0, e_sb[:],
                    op0=ALU.add, op1=ALU.min)

            def v_bundle(tb, j):
                xt = xt_tiles[tb]
                jj = (TB // SB) * tb + j
                vps = av.tile([128, 256], F32, tag="av", name="v_ps")
                nc.tensor.matmul(vps[:], ones1[:], bv_sb[:],
                                 start=True, stop=False)
                for k in range(KD):
                    nc.tensor.matmul(
                        vps[:],
                        xt[:, TB * k + SB * j:TB * k + SB * (j + 1)],
                        wv_sb[:, 256 * k:256 * (k + 1)],
                        start=False, stop=(k == KD - 1))
                # V psum [128t, 4h x 64g] -> vkT V columns (strided)
                vdst = bass.AP(tensor=vkT.tensor,
                               offset=vkT.offset + 512 * jj,
                               ap=[vkT.ap[0], [128, 4], [1, G]])
                vsrc = bass.AP(tensor=vps.tensor, offset=vps.offset,
                               ap=[vps.ap[0], [G, 4], [1, G]])
                nc.scalar.activation(vdst, vsrc, AF.Copy)
                # K^T 4-heads one transpose -> vkT K columns (strided)
                ktp = av.tile([128, 128], BF16, tag="av", name="kt_ps")
                nc.tensor.transpose(ktp[:], kT[:, SB * jj:SB * (jj + 1)],
                                    ident[:])
                kdst = bass.AP(tensor=vkT.tensor,
                               offset=vkT.offset + 512 * jj + G,
                               ap=[vkT.ap[0], [128, 4], [1, F]])
                ksrc = bass.AP(tensor=ktp.tensor, offset=ktp.offset,
                               ap=[ktp.ap[0], [F, 4], [1, F]])
                nc.vector.tensor_copy(kdst, ksrc)

            def proj_bundles(tb):
                return [
                    lambda tb=tb: qk_bundle(tb, wq_sb, bq_sb, qT),
                    lambda tb=tb: qk_bundle(tb, wk_sb, bk_sb, kT),
                    lambda tb=tb, j=0: v_bundle(tb, j),
                    lambda tb=tb, j=1: v_bundle(tb, j),
                    lambda tb=tb, j=2: v_bundle(tb, j),
                    lambda tb=tb, j=3: v_bundle(tb, j),
                ]

            def junk_warm():
                # dummy fat matmul: keeps the PE MAC duty over the HAM
                # un-throttle threshold in chunks with no projection left
                jk = bcp.tile([128, 512], F32, tag="bc", name="jk_ps")
                nc.tensor.matmul(jk[:], ident[:], qT[:, 0:512],
                                 start=True, stop=True)

            # ---- attention, feature-major output, head-pair banks ----
            s_bf = [None]     # bf16 copy of running state (all 4 heads)
            s_all = sp.tile([128, 128], F32, tag="s", name="s_all")

            def chunk_attn(i, fill=()):
                fill = list(fill)
                t0 = CHUNK * i
                csl = slice(t0, t0 + CHUNK)
                # intra-chunk A^T = K_s . Q_t, masked, per head
                ams = []
                for h in range(4):
                    fsl = slice(32 * h, 32 * (h + 1))
                    tp = (32 * h, 0) if h == 3 else None
                    a_ps = av.tile([128, 384], F32, tag="av", name="a_ps")
                    nc.tensor.matmul(a_ps[:, 0:CHUNK],
                                     kT[fsl, t0:t0 + 128], qT[fsl, csl],
                                     start=True, stop=True, tile_position=tp)
                    nc.tensor.matmul(a_ps[:, CHUNK:384],
                                     kT[fsl, t0 + 128:t0 + 256],
                                     qT[fsl, t0 + 128:t0 + 256],
                                     start=True, stop=True, tile_position=tp)
                    am = work.tile([128, 384], BF16, tag="am", name="am")
                    nc.vector.tensor_tensor(am[:], a_ps[:],
                                            maskc_sb[:, 0:384], op=ALU.mult)
                    ams.append(am)
                if fill:
                    fill.pop(0)()
                # f-major o per head pair: [V-rows 0:64 | junk | den row 96],
                # head-even at cols 0:256, head-odd at 256:512
                for p in range(2):
                    o_ps = op.tile([128, 512], F32, tag="o", name="o_ps")
                    for hp in range(2):
                        h = 2 * p + hp
                        fsl = slice(32 * h, 32 * (h + 1))
                        am = ams[h]
                        osl = o_ps[:, 256 * hp:256 * (hp + 1)]
                        nc.tensor.matmul(osl[0:97, :], vk_blk(2 * i, h, 0, 97),
                                         am[:, 0:CHUNK],
                                         start=True, stop=False)
                        nc.tensor.matmul(
                            osl[0:97, 128:256], vk_blk(2 * i + 1, h, 0, 97),
                            am[:, CHUNK:384],
                            start=False, stop=(s_bf[0] is None))
                        if s_bf[0] is not None:
                            nc.tensor.matmul(
                                osl[0:97, :], s_bf[0][fsl, 0:97],
                                qT[fsl, csl], start=False, stop=True,
                                tile_position=(32 * h, 0) if h == 3 else None)
                    # normalize both heads: PE-broadcast the den row to 64
                    # rows, reciprocal (PSUM->SBUF), two tensor multiplies
                    den_r = work.tile([1, 512], mybir.dt.float32r, tag="den",
                                      name="den_r")
                    nc.scalar.activation(den_r[:], o_ps[96:97, :], AF.Copy)
                    bc_ps = bcp.tile([64, 512], F32, tag="bc", name="bc_ps")
                    nc.tensor.matmul(bc_ps[:], onesr[:], den_r[:],
                                     start=True, stop=True)
                    rec_bc = work.tile([64, 512], F32, tag="rec", name="rec_bc")
                    nc.vector.reciprocal_approx_fast(rec_bc[:], bc_ps[:])
                    nc.vector.tensor_tensor(
                        aT[0:G, N * p + t0:N * p + t0 + CHUNK],
                        o_ps[0:G, 0:256], rec_bc[:, 0:256], op=ALU.mult)
                    nc.vector.tensor_tensor(
                        aT[G:2 * G, N * p + t0:N * p + t0 + CHUNK],
                        o_ps[0:G, 256:512], rec_bc[:, 256:512], op=ALU.mult)
                    if fill:
                        fill.pop(0)()
                # state S += K_chunk^T [V|K|ones] (PSUM-resident accumulate)
                if i < NCHUNK - 1:
                    for h in range(4):
                        ssl = s_all[32 * h:32 * (h + 1), :]
                        tp = (0, 32 * h) if h == 3 else None
                        nc.tensor.matmul(ssl, vk_blk(2 * i, h, G, G + F),
                                         vk_blk(2 * i, h),
                                         start=(i == 0), stop=False,
                                         skip_group_check=True,
                                         tile_position=tp)
                        nc.tensor.matmul(ssl, vk_blk(2 * i + 1, h, G, G + F),
                                         vk_blk(2 * i + 1, h),
                                         start=False, stop=True,
                                         skip_group_check=True,
                                         tile_position=tp)
                    sb16 = sbf16p.tile([128, 128], BF16, tag="sbf",
                                       name="sb16")
                    nc.vector.tensor_copy(sb16[:], s_all[:])
                    s_bf[0] = sb16
                for f in fill:
                    f()

            def yproj_half(i, half):
                # output projection for one 128-t tile of chunk i
                tt = 2 * i + half
                tsl = slice(128 * tt, 128 * (tt + 1))
                for eb in range(2):
                    esl = slice(512 * eb, 512 * (eb + 1))
                    y_ps = pp.tile([128, 512], F32, tag="p", name="y_ps")
                    for j in range(2):
                        nc.tensor.matmul(
                            y_ps[:], aT[:, N * j:N * (j + 1)][:, tsl],
                            wo_sb[:, D * j:D * j + D][:, esl],
                            start=(j == 0), stop=(j == 1))
                    y_sb = ysb.tile([128, 512], BF16, tag="ysb",
                                    name="y_sb")
                    if (tt + eb) % 2 == 0:
                        nc.scalar.activation(y_sb[:], y_ps[:], AF.Copy)
                    else:
                        nc.vector.tensor_copy(y_sb[:], y_ps[:])
                    nc.sync.dma_start(y_e[tt, eb], y_sb[:])

            # ---- schedule: minimal projections up front; the rest of the
            # projection bundles, the deferred yproj halves and junk warmers
            # fill slots inside each chunk so every HAM window keeps fat
            # N=512 PE streams.  Chunk i consumes vk s-blocks 2i,2i+1 which
            # its predecessors' V-bundles produce; a-matmuls of chunk i need
            # the K/Q bundles of t-block i//2 (emitted >= 1 chunk earlier).
            B_ = {}
            for tb in range(4):
                bl = proj_bundles(tb)
                for nm, bb in zip(("Q", "K", "V0", "V1", "V2", "V3"), bl):
                    B_[f"{nm}{tb}"] = bb

            def yh(i, half):
                return lambda: yproj_half(i, half)

            for nm in ("Q0", "K0", "V00", "V10"):
                B_[nm]()
            fills = [
                ["V20", "V30", "Q1", junk_warm],
                ["K1", "V01", "V11", yh(0, 0), yh(0, 1)],
                ["V21", "V31", "Q2", yh(1, 0), yh(1, 1)],
                ["K2", "V02", "V12", yh(2, 0), yh(2, 1)],
                ["V22", "V32", "Q3", yh(3, 0), yh(3, 1)],
                ["K3", "V03", "V13", yh(4, 0), yh(4, 1)],
                ["V23", "V33", junk_warm, yh(5, 0), yh(5, 1)],
                [junk_warm, junk_warm, yh(6, 0), yh(6, 1), junk_warm],
            ]
            for i in range(NCHUNK):
                fl = [B_[f] if isinstance(f, str) else f for f in fills[i]]
                chunk_attn(i, fill=fl)
            yproj_half(7, 0)
            yproj_half(7, 1)

    nc.compile()
    return nc


def make_in_maps(x, wq, bq, wk, bk, wv, bv, wo, bo):
    x = np.asarray(x, np.float32)
    in_maps = []
    for c in range(NCORES):
        b, r = divmod(c, 4)
        xt_b = x[b].T.astype(BF16NP)                    # (D, N)
        xt_tiled = np.ascontiguousarray(
            xt_b.reshape(D // 128, 128, N // TB, TB).transpose(2, 0, 1, 3))
        wq_s = np.asarray(wq)[:, 128 * r:128 * (r + 1)].astype(BF16NP)
        wk_s = np.asarray(wk)[:, 128 * r:128 * (r + 1)].astype(BF16NP)
        wv_s = np.asarray(wv)[:, 256 * r:256 * (r + 1)].astype(BF16NP)
        wo_s = np.asarray(wo)[256 * r:256 * (r + 1), :].astype(BF16NP)
        in_maps.append({
            "xT": xt_tiled,
            "wq": np.ascontiguousarray(wq_s.reshape(D // 128, 128, 4 * F)),
            "wk": np.ascontiguousarray(wk_s.reshape(D // 128, 128, 4 * F)),
            "wv": np.ascontiguousarray(
                wv_s.reshape(D // 128, 128, 2, 128).transpose(0, 2, 1, 3)
            ).reshape(D // 128 * 2, 128, 128),
            "wo": np.ascontiguousarray(wo_s.reshape(2, 128, D)),
            "bq": np.ascontiguousarray(
                np.asarray(bq)[128 * r:128 * (r + 1)],
                dtype=np.float32).reshape(-1, 1),
            "bk": np.ascontiguousarray(
                np.asarray(bk)[128 * r:128 * (r + 1)],
                dtype=np.float32).reshape(-1, 1),
            "bv": np.ascontiguousarray(
                np.asarray(bv)[256 * r:256 * (r + 1)].astype(BF16NP)
            ).reshape(1, -1),
        })
    return in_maps


def assemble(results, bo):
    y = np.zeros((B, N, D), np.float32)
    for c in range(NCORES):
        yt = results[c]["y"].astype(np.float32)   # (N//128, 2, 128, 512)
        y[c // 4] += yt.transpose(0, 2, 1, 3).reshape(N, D)
    return y + np.asarray(bo, np.float32).reshape(1, 1, D)


_NC_CACHE = {}


def run(inputs, trace=False):
    _install_ntff_hook()
    from concourse.bass_utils import run_bass_kernel_spmd
    if "nc" not in _NC_CACHE:
        _NC_CACHE["nc"] = build_nc()
    nc = _NC_CACHE["nc"]
    in_maps = make_in_maps(**inputs)
    res = run_bass_kernel_spmd(nc, in_maps, core_ids=list(range(NCORES)),
                               trace=trace)
    return assemble(res.results, inputs["bo"]), res.exec_time_ns


def kernel(**inputs) -> np.ndarray:
    y, _ = run(inputs, trace=False)
    return y


# revision 42
# speedup vs baseline: 1.1861x; 1.0350x over previous
"""Trainium2 Bass kernel for chunked causal linear attention (elu+1 feature map).

Reference computation (B=2, N=2048, D=1024, DHAT=512, H=16, F=32, G=64):
    Q = phi(x @ wq + bq), K = phi(x @ wk + bk), V = x @ wv + bv   (per-head split)
    kv_t = cumsum_t(K_t outer V_t);  Z_t = 1/(Q_t . cumsum_t(K)_t + 1e-6)
    out_t = (Q_t . kv_t) * Z_t;  y = out @ wo + bo
with phi(u) = elu(u) + 1 = min(exp(u), max(u + 1, 1)).

Sharding over 8 cores: core c handles batch b = c//4 and heads 4r..4r+3
(r = c%4).  Each core projects its head slice, runs chunk-parallel linear
attention (chunk C=256), and computes a PARTIAL output projection through its
256 rows of wo; the host sums the 4 bf16 partials per batch plus bo.

Layout strategy (v2):
  * Projection phase builds, per 512-t block: feature-major Q^T/K^T (for the
    intra-chunk A matmuls), and t-major [V|K|ones] 128-t blocks in one
    persistent SBUF tile vkT (V projected DIRECTLY t-major with bias folded
    in as a rank-1 matmul; K transposed 4-heads-at-a-time on the PE).
  * Attention is computed t-major: per 128-t block the PE produces
    o[t, 4 x (V,.,den)] so the denominator lands as a per-partition column;
    normalize is then a per-partition-scale ACT op (no PE broadcast, no wide
    DVE reciprocal/multiply).  The normalized attn is transposed back to
    feature-major for the output projection (2 PE transposes / 128-t block).
  * Running state S = cumsum K^T[V|K|ones] stays RESIDENT IN PSUM (PE
    accumulates across chunks); only a [128,128] bf16 cast per chunk runs on
    the DVE.
  * y partials are written bf16 (halves output DMA).
"""
import os
import sys
import types

sys.path.insert(0, "/opt/trn_rl_repo")

import ml_dtypes
import numpy as np

# ---- problem constants (hardcoded; kernel.py must be self-contained) ----
B, N, D, DHAT, H = 2, 2048, 1024, 512, 16
F = DHAT // H        # 32
G = D // H           # 64
NCORES = 8
CHUNK = 256          # attention chunk along t
NCHUNK = N // CHUNK  # 8
SB = 128             # s-block (128-t block)
NSB = N // SB        # 16
TB = 512             # projection t-block
KD = D // 128        # 8 contraction tiles
BF16NP = ml_dtypes.bfloat16


def _install_ntff_hook():
    """Register the axon NTFF profiling hook (stub antenv lacks axon_hooks)."""
    if "antenv.axon_hooks" in sys.modules:
        return
    try:
        from trn_agent_boot.trn_boot import _ntff_profile_via_ctypes
        hook = _ntff_profile_via_ctypes("/opt/axon/libaxon_pjrt.so")
    except Exception:
        hook = None
    m = types.ModuleType("antenv.axon_hooks")
    m.get_axon_ntff_profile_hook = lambda: hook
    m.set_axon_ntff_profile_hook = lambda h: None
    sys.modules["antenv.axon_hooks"] = m


def build_nc(with_bv=True):
    import concourse.bass as bass
    import concourse.mybir as mybir
    import concourse.tile as tile
    from concourse import bacc

    F32 = mybir.dt.float32
    BF16 = mybir.dt.bfloat16
    AF = mybir.ActivationFunctionType
    ALU = mybir.AluOpType

    nc = bacc.Bacc("TRN2", target_bir_lowering=False, debug=False,
                   num_devices=NCORES)

    # ---- per-core DRAM parameters (bf16 operands, pre-tiled on host) ----
    xT_e = nc.declare_dram_parameter("xT", [N // TB, KD, 128, TB],
                                     BF16, isOutput=False)
    wq_e = nc.declare_dram_parameter("wq", [KD, 128, 4 * F], BF16,
                                     isOutput=False)
    wk_e = nc.declare_dram_parameter("wk", [KD, 128, 4 * F], BF16,
                                     isOutput=False)
    wv_e = nc.declare_dram_parameter("wv", [KD * 2, 128, 128], BF16,
                                     isOutput=False)
    wo_e = nc.declare_dram_parameter("wo", [2, 128, D], BF16, isOutput=False)
    bq_e = nc.declare_dram_parameter("bq", [4 * F, 1], F32, isOutput=False)
    bk_e = nc.declare_dram_parameter("bk", [4 * F, 1], F32, isOutput=False)
    bv_e = nc.declare_dram_parameter("bv", [1, 4 * G], BF16, isOutput=False)
    y_e = nc.declare_dram_parameter("y", [N // 128, 2, 128, 512], BF16,
                                    isOutput=True)

    # causal mask [triu(s0 vs t) | triu(s1 vs t-high)] for one 256-chunk:
    # cols 0:256 mask block0 [s0, t 0:256]; cols 256:384 mask block1
    # [s1, t 128:256]
    m0 = np.zeros((128, CHUNK), np.float32)
    tri = np.zeros((128, 128), np.float32)
    for s in range(128):
        m0[s, s:] = 1.0
        tri[s, s:] = 1.0
    mask1 = np.concatenate([m0, tri], axis=1)          # [128, 384]
    maskc_d = nc.inline_tensor(
        np.concatenate([mask1, mask1], axis=1).astype(BF16NP), "maskc")
    ident_d = nc.inline_tensor(np.eye(128, dtype=np.float32).astype(BF16NP),
                               "identc")
    ones1_d = nc.inline_tensor(np.ones((1, 128), np.float32).astype(BF16NP),
                               "ones1c")
    onesr_d = nc.inline_tensor(np.ones((1, 64), np.float32), "onesrc")

    with tile.TileContext(nc) as tc:
        with (
            tc.tile_pool(name="persist", bufs=1) as pers,
            tc.tile_pool(name="xin", bufs=4) as xin,
            tc.tile_pool(name="ppool", bufs=2, space="PSUM") as pp,
            tc.tile_pool(name="avpool", bufs=2, space="PSUM") as av,
            tc.tile_pool(name="opool", bufs=2, space="PSUM") as op,
            tc.tile_pool(name="bcpool", bufs=1, space="PSUM") as bcp,
            tc.tile_pool(name="spool", bufs=1, space="PSUM") as sp,
            tc.tile_pool(name="work", bufs=6) as work,
            tc.tile_pool(name="sbf16", bufs=2) as sbf16p,
            tc.tile_pool(name="ysb", bufs=3) as ysb,
        ):
            # ---- persistent SBUF tiles ----
            wq_sb = pers.tile([128, KD * 128], BF16)
            wk_sb = pers.tile([128, KD * 128], BF16)
            wv_sb = pers.tile([128, KD * 256], BF16)
            wo_sb = pers.tile([128, 2 * D], BF16)
            bq_sb = pers.tile([4 * F, 1], F32)
            bk_sb = pers.tile([4 * F, 1], F32)
            bv_sb = pers.tile([1, 4 * G], BF16)
            ones1 = pers.tile([1, 128], BF16)
            onesr = pers.tile([1, 64], mybir.dt.float32r)
            ident = pers.tile([128, 128], BF16)
            maskc_sb = pers.tile([128, 768], BF16)
            qT = pers.tile([128, N], BF16)
            kT = pers.tile([128, N], BF16)
            # t-major [V|K|ones] blocks: s-block j at cols 512j, head h at
            # 512j+128h: [V(0:64) | K(64:96) | ones(96) | zero(97:128)]
            vkT = pers.tile([128, NSB * 512], BF16)
            # feature-major normalized attn: j-half jh (heads 2jh,2jh+1) at
            # cols jh*N + t
            aT = pers.tile([128, 2 * N], BF16)

            # zero vkT, then set the ones columns (col 96 of each block)
            nc.vector.memset(vkT[:], 0.0)
            nc.vector.memset(
                bass.AP(tensor=vkT.tensor, offset=vkT.offset + 96,
                        ap=[vkT.ap[0], [128, 4 * NSB]]), 1.0)

            # ---- startup DMAs: interleave weight/x issue on 2 engines so
            # the first projection matmul starts ~2us in ----
            # sync engine: x t-block 0 (2 halves), ident, x prefetches, wo
            xt_tiles = []
            xt0 = xin.tile([128, KD * TB], BF16, tag="xt", name="xt0")
            for q in range(4):
                nc.sync.dma_start(
                    xt0[:, 2 * q * TB:2 * (q + 1) * TB].rearrange(
                        "p (k c) -> p k c", k=2),
                    xT_e[0, 2 * q:2 * q + 2].rearrange("k p c -> p k c"))
            xt_tiles.append(xt0)
            nc.sync.dma_start(ident[:], ident_d[:])
            # scalar engine: weights + biases (issue in first-use order)
            for hh in range(2):
                nc.scalar.dma_start(
                    wq_sb[:, 512 * hh:512 * (hh + 1)].rearrange(
                        "p (k c) -> p k c", k=4),
                    wq_e[4 * hh:4 * (hh + 1)].rearrange("k p c -> p k c"))
            nc.scalar.dma_start(
                wk_sb[:].rearrange("p (k c) -> p k c", k=KD),
                wk_e[:].rearrange("k p c -> p k c"))
            nc.scalar.dma_start(bq_sb[:], bq_e[:])
            nc.scalar.dma_start(bk_sb[:], bk_e[:])
            nc.scalar.dma_start(
                wv_sb[:].rearrange("p (km c) -> p km c", km=KD * 2),
                wv_e[:].rearrange("km p c -> p km c"))
            nc.scalar.dma_start(bv_sb[:], bv_e[:])
            nc.scalar.dma_start(ones1[:], ones1_d[:])
            nc.scalar.dma_start(onesr[:], onesr_d[:].bitcast(mybir.dt.float32r))

            # remaining prefetches (sync engine), consts needed by chunk 0
            # issued before the later x blocks
            xt1 = xin.tile([128, KD * TB], BF16, tag="xt", name="xt1")
            nc.sync.dma_start(
                xt1[:].rearrange("p (k c) -> p k c", k=KD),
                xT_e[1].rearrange("k p c -> p k c"))
            xt_tiles.append(xt1)
            nc.sync.dma_start(maskc_sb[:], maskc_d[:])
            nc.sync.dma_start(
                wo_sb[:].rearrange("p (j c) -> p j c", j=2),
                wo_e[:].rearrange("j p c -> p j c"))
            for tb in range(2, N // TB):
                xt = xin.tile([128, KD * TB], BF16, tag="xt", name="xt")
                nc.sync.dma_start(
                    xt[:].rearrange("p (k c) -> p k c", k=KD),
                    xT_e[tb].rearrange("k p c -> p k c"))
                xt_tiles.append(xt)

            def vk_blk(jj, h, c0=0, c1=128):
                return bass.AP(
                    tensor=vkT.tensor,
                    offset=vkT.offset + 512 * jj + 128 * h + c0,
                    ap=[vkT.ap[0], [1, c1 - c0]])

            # ---- projection phase, emitted as bundles so t-blocks 1..3
            # interleave into the attention chunks (their fat N=512 streams
            # keep the PE HAM clock-gate at full rate) ----
            def qk_bundle(tb, w_sb, b_sb, dstT):
                tsl = slice(TB * tb, TB * (tb + 1))
                xt = xt_tiles[tb]
                ps = pp.tile([128, TB], F32, tag="p", name="qk_ps")
                for k in range(KD):
                    nc.tensor.matmul(ps[:], w_sb[:, 128 * k:128 * (k + 1)],
                                     xt[:, TB * k:TB * (k + 1)],
                                     start=(k == 0), stop=(k == KD - 1))
                # phi(u) = min(exp(u), relu(u) + 1), bias folded in
                e_sb = work.tile([128, TB], BF16, tag="phi_e", name="phi_e")
                nc.scalar.activation(e_sb[:], ps[:], AF.Exp, bias=b_sb[:])
                r_sb = work.tile([128, TB], BF16, tag="phi_r", name="phi_r")
                nc.scalar.activation(r_sb[:], ps[:], AF.Relu, bias=b_sb[:])
                nc.vector.scalar_tensor_tensor(
                    dstT[:, tsl], r_sb[:], 1.0, e_sb[:],
                    op0=ALU.add, op1=ALU.min)

            def v_bundle(tb, j):
                xt = xt_tiles[tb]
                jj = (TB // SB) * tb + j
                vps = av.tile([128, 256], F32, tag="av", name="v_ps")
                if with_bv:
                    nc.tensor.matmul(vps[:], ones1[:], bv_sb[:],
                                     start=True, stop=False)
                for k in range(KD):
                    nc.tensor.matmul(
                        vps[:],
                        xt[:, TB * k + SB * j:TB * k + SB * (j + 1)],
                        wv_sb[:, 256 * k:256 * (k + 1)],
                        start=(not with_bv and k == 0), stop=(k == KD - 1))
                # V psum [128t, 4h x 64g] -> vkT V columns (strided)
                vdst = bass.AP(tensor=vkT.tensor,
                               offset=vkT.offset + 512 * jj,
                               ap=[vkT.ap[0], [128, 4], [1, G]])
                vsrc = bass.AP(tensor=vps.tensor, offset=vps.offset,
                               ap=[vps.ap[0], [G, 4], [1, G]])
                nc.scalar.activation(vdst, vsrc, AF.Copy)
                # K^T 4-heads one transpose -> vkT K columns (strided)
                ktp = av.tile([128, 128], BF16, tag="av", name="kt_ps")
                nc.tensor.transpose(ktp[:], kT[:, SB * jj:SB * (jj + 1)],
                                    ident[:])
                kdst = bass.AP(tensor=vkT.tensor,
                               offset=vkT.offset + 512 * jj + G,
                               ap=[vkT.ap[0], [128, 4], [1, F]])
                ksrc = bass.AP(tensor=ktp.tensor, offset=ktp.offset,
                               ap=[ktp.ap[0], [F, 4], [1, F]])
                nc.vector.tensor_copy(kdst, ksrc)

            def proj_bundles(tb):
                return [
                    lambda tb=tb: qk_bundle(tb, wq_sb, bq_sb, qT),
                    lambda tb=tb: qk_bundle(tb, wk_sb, bk_sb, kT),
                    lambda tb=tb, j=0: v_bundle(tb, j),
                    lambda tb=tb, j=1: v_bundle(tb, j),
                    lambda tb=tb, j=2: v_bundle(tb, j),
                    lambda tb=tb, j=3: v_bundle(tb, j),
                ]

            def junk_warm():
                # dummy fat matmul: keeps the PE MAC duty over the HAM
                # un-throttle threshold in chunks with no projection left
                jk = bcp.tile([128, 512], F32, tag="bc", name="jk_ps")
                nc.tensor.matmul(jk[:], ident[:], qT[:, 0:512],
                                 start=True, stop=True)

            # ---- attention, feature-major output, head-pair banks ----
            s_bf = [None]     # bf16 copy of running state (all 4 heads)
            s_all = sp.tile([128, 128], F32, tag="s", name="s_all")

            def chunk_attn(i, fill=()):
                fill = list(fill)
                t0 = CHUNK * i
                csl = slice(t0, t0 + CHUNK)
                # intra-chunk A^T = K_s . Q_t, masked, per head
                ams = []
                for h in range(4):
                    fsl = slice(32 * h, 32 * (h + 1))
                    tp = (32 * h, 0) if h == 3 else None
                    a_ps = av.tile([128, 384], F32, tag="av", name="a_ps")
                    nc.tensor.matmul(a_ps[:, 0:CHUNK],
                                     kT[fsl, t0:t0 + 128], qT[fsl, csl],
                                     start=True, stop=True, tile_position=tp)
                    nc.tensor.matmul(a_ps[:, CHUNK:384],
                                     kT[fsl, t0 + 128:t0 + 256],
                                     qT[fsl, t0 + 128:t0 + 256],
                                     start=True, stop=True, tile_position=tp)
                    am = work.tile([128, 384], BF16, tag="am", name="am")
                    nc.vector.tensor_tensor(am[:], a_ps[:],
                                            maskc_sb[:, 0:384], op=ALU.mult)
                    ams.append(am)
                if fill:
                    fill.pop(0)()
                # f-major o per head pair: [V-rows 0:64 | junk | den row 96],
                # head-even at cols 0:256, head-odd at 256:512
                for p in range(2):
                    o_ps = op.tile([128, 512], F32, tag="o", name="o_ps")
                    for hp in range(2):
                        h = 2 * p + hp
                        fsl = slice(32 * h, 32 * (h + 1))
                        am = ams[h]
                        osl = o_ps[:, 256 * hp:256 * (hp + 1)]
                        nc.tensor.matmul(osl[0:97, :], vk_blk(2 * i, h, 0, 97),
                                         am[:, 0:CHUNK],
                                         start=True, stop=False)
                        nc.tensor.matmul(
                            osl[0:97, 128:256], vk_blk(2 * i + 1, h, 0, 97),
                            am[:, CHUNK:384],
                            start=False, stop=(s_bf[0] is None))
                        if s_bf[0] is not None:
                            nc.tensor.matmul(
                                osl[0:97, :], s_bf[0][fsl, 0:97],
                                qT[fsl, csl], start=False, stop=True,
                                tile_position=(32 * h, 0) if h == 3 else None)
                    # normalize both heads: PE-broadcast the den row to 64
                    # rows, reciprocal (PSUM->SBUF), two tensor multiplies
                    den_r = work.tile([1, 512], mybir.dt.float32r, tag="den",
                                      name="den_r")
                    nc.scalar.activation(den_r[:], o_ps[96:97, :], AF.Copy)
                    bc_ps = bcp.tile([64, 512], F32, tag="bc", name="bc_ps")
                    nc.tensor.matmul(bc_ps[:], onesr[:], den_r[:],
                                     start=True, stop=True)
                    rec_bc = work.tile([64, 512], F32, tag="rec", name="rec_bc")
                    nc.vector.reciprocal_approx_fast(rec_bc[:], bc_ps[:])
                    nc.vector.tensor_tensor(
                        aT[0:G, N * p + t0:N * p + t0 + CHUNK],
                        o_ps[0:G, 0:256], rec_bc[:, 0:256], op=ALU.mult)
                    nc.vector.tensor_tensor(
                        aT[G:2 * G, N * p + t0:N * p + t0 + CHUNK],
                        o_ps[0:G, 256:512], rec_bc[:, 256:512], op=ALU.mult)
                    if fill:
                        fill.pop(0)()
                # state S += K_chunk^T [V|K|ones] (PSUM-resident accumulate);
                # s-block-major so the four 32-col-group matmuls overlap
                if i < NCHUNK - 1:
                    for sb2 in range(2):
                        for h in range(4):
                            ssl = s_all[32 * h:32 * (h + 1), :]
                            tp = (0, 32 * h) if h == 3 else None
                            nc.tensor.matmul(
                                ssl, vk_blk(2 * i + sb2, h, G, G + F),
                                vk_blk(2 * i + sb2, h),
                                start=(i == 0 and sb2 == 0),
                                stop=(sb2 == 1),
                                skip_group_check=True, tile_position=tp)
                    sb16 = sbf16p.tile([128, 128], BF16, tag="sbf",
                                       name="sb16")
                    nc.vector.tensor_copy(sb16[:], s_all[:])
                    s_bf[0] = sb16
                for f in fill:
                    f()

            def yproj_half(i, half):
                # output projection for one 128-t tile of chunk i
                tt = 2 * i + half
                tsl = slice(128 * tt, 128 * (tt + 1))
                for eb in range(2):
                    esl = slice(512 * eb, 512 * (eb + 1))
                    y_ps = pp.tile([128, 512], F32, tag="p", name="y_ps")
                    for j in range(2):
                        nc.tensor.matmul(
                            y_ps[:], aT[:, N * j:N * (j + 1)][:, tsl],
                            wo_sb[:, D * j:D * j + D][:, esl],
                            start=(j == 0), stop=(j == 1))
                    y_sb = ysb.tile([128, 512], BF16, tag="ysb",
                                    name="y_sb")
                    if (tt + eb) % 2 == 0:
                        nc.scalar.activation(y_sb[:], y_ps[:], AF.Copy)
                    else:
                        nc.vector.tensor_copy(y_sb[:], y_ps[:])
                    nc.sync.dma_start(y_e[tt, eb], y_sb[:])

            # ---- schedule: minimal projections up front; the rest of the
            # projection bundles, the deferred yproj halves and junk warmers
            # fill slots inside each chunk so every HAM window keeps fat
            # N=512 PE streams.  Chunk i consumes vk s-blocks 2i,2i+1 which
            # its predecessors' V-bundles produce; a-matmuls of chunk i need
            # the K/Q bundles of t-block i//2 (emitted >= 1 chunk earlier).
            B_ = {}
            for tb in range(4):
                bl = proj_bundles(tb)
                for nm, bb in zip(("Q", "K", "V0", "V1", "V2", "V3"), bl):
                    B_[f"{nm}{tb}"] = bb

            def yh(i, half):
                return lambda: yproj_half(i, half)

            for nm in ("Q0", "K0", "V00", "V10"):
                B_[nm]()
            fills = [
                ["V20", "V30", "Q1", junk_warm],
                ["K1", "V01", "V11", yh(0, 0), yh(0, 1)],
                ["V21", "V31", "Q2", yh(1, 0), yh(1, 1)],
                ["K2", "V02", "V12", yh(2, 0), yh(2, 1)],
                ["V22", "V32", "Q3", yh(3, 0), yh(3, 1)],
                ["K3", "V03", "V13", yh(4, 0), yh(4, 1)],
                ["V23", "V33", junk_warm, yh(5, 0), yh(5, 1)],
                [junk_warm, junk_warm, yh(6, 0), yh(6, 1), junk_warm],
            ]
            for i in range(NCHUNK):
                fl = [B_[f] if isinstance(f, str) else f for f in fills[i]]
                chunk_attn(i, fill=fl)
            yproj_half(7, 0)
            yproj_half(7, 1)

    nc.compile()
    return nc


def make_in_maps(x, wq, bq, wk, bk, wv, bv, wo, bo):
    x = np.asarray(x, np.float32)
    in_maps = []
    for c in range(NCORES):
        b, r = divmod(c, 4)
        xt_b = x[b].T.astype(BF16NP)                    # (D, N)
        xt_tiled = np.ascontiguousarray(
            xt_b.reshape(D // 128, 128, N // TB, TB).transpose(2, 0, 1, 3))
        wq_s = np.asarray(wq)[:, 128 * r:128 * (r + 1)].astype(BF16NP)
        wk_s = np.asarray(wk)[:, 128 * r:128 * (r + 1)].astype(BF16NP)
        wv_s = np.asarray(wv)[:, 256 * r:256 * (r + 1)].astype(BF16NP)
        wo_s = np.asarray(wo)[256 * r:256 * (r + 1), :].astype(BF16NP)
        in_maps.append({
            "xT": xt_tiled,
            "wq": np.ascontiguousarray(wq_s.reshape(D // 128, 128, 4 * F)),
            "wk": np.ascontiguousarray(wk_s.reshape(D // 128, 128, 4 * F)),
            "wv": np.ascontiguousarray(
                wv_s.reshape(D // 128, 128, 2, 128).transpose(0, 2, 1, 3)
            ).reshape(D // 128 * 2, 128, 128),
            "wo": np.ascontiguousarray(wo_s.reshape(2, 128, D)),
            "bq": np.ascontiguousarray(
                np.asarray(bq)[128 * r:128 * (r + 1)],
                dtype=np.float32).reshape(-1, 1),
            "bk": np.ascontiguousarray(
                np.asarray(bk)[128 * r:128 * (r + 1)],
                dtype=np.float32).reshape(-1, 1),
            "bv": np.ascontiguousarray(
                np.asarray(bv)[256 * r:256 * (r + 1)].astype(BF16NP)
            ).reshape(1, -1),
        })
    return in_maps


def assemble(results, bo):
    y = np.zeros((B, N, D), np.float32)
    for c in range(NCORES):
        yt = results[c]["y"].astype(np.float32)   # (N//128, 2, 128, 512)
        y[c // 4] += yt.transpose(0, 2, 1, 3).reshape(N, D)
    return y + np.asarray(bo, np.float32).reshape(1, 1, D)


_NC_CACHE = {}


def run(inputs, trace=False):
    _install_ntff_hook()
    from concourse.bass_utils import run_bass_kernel_spmd
    with_bv = bool(np.any(np.asarray(inputs["bv"], np.float32)))
    key = ("nc", with_bv)
    if key not in _NC_CACHE:
        _NC_CACHE[key] = build_nc(with_bv=with_bv)
    nc = _NC_CACHE[key]
    in_maps = make_in_maps(**inputs)
    res = run_bass_kernel_spmd(nc, in_maps, core_ids=list(range(NCORES)),
                               trace=trace)
    return assemble(res.results, inputs["bo"]), res.exec_time_ns


def kernel(**inputs) -> np.ndarray:
    y, _ = run(inputs, trace=False)
    return y
